# revision 19
# baseline (speedup 1.0000x reference)
"""Trainium2 Bass kernel for LocalXLAttention (chunk-summed variant).

Math: the reference einsum sums over the chunk index z, so every query
attends to the same three [w, dh] K/V matrices built from chunk sums:
  K_prev = S_k - k_chunk[C-1], K_cur = S_k, K_next = S_k - k_chunk[0]
(and identically for V), where S_k = sum_c k_chunk[c].  The computation
collapses to, per sequence position l and head h:
  attn[l,h,:]  = qp[l,h,:] @ KbigT          (KbigT: [dh, 3w])
  probs        = softmax(attn, axis=-1)
  ctx[l,h,:]   = probs[l,h,:] @ Vbig        (Vbig:  [3w, dh])
  out          = ctx.reshape(L, dm) @ Wc

Sharding: L=4096 is split 512 rows per core across 8 NeuronCores
(data-parallel over the sequence; no collectives).  Each core redundantly
computes the tiny chunk-summed K/V from the full kv input.

All matmul operands are bf16 (fp32 PSUM accumulation).  exp runs on the
scalar engine directly from PSUM (its 1 elem/cycle/lane rate is the
kernel's floor); the attention loop is organized so ACT never waits:
even PSUM ring parity across pairs, QP_T chunks and the deferred softmax
normalization ride in the PE/DVE slack.  Inputs load as one large DMA
per tensor (the ~2us fixed cost per dma_start dominates small
transfers), spread across all five trigger-engine rings.
"""

import sys
for _p in ('/opt/pypackages', '/opt/trn_rl_repo'):
    if _p not in sys.path:
        sys.path.insert(0, _p)

import numpy as np
import ml_dtypes

import concourse.bass as bass
import concourse.bacc as bacc
import concourse.tile as tile
from concourse import mybir
from concourse.bass_utils import run_bass_kernel_spmd
from concourse.masks import make_identity

F32 = mybir.dt.float32
BF16 = mybir.dt.bfloat16
AF = mybir.ActivationFunctionType

N_CORES = 8
L = 4096          # full sequence
LS = L // N_CORES # 512 rows per core
DM = 1024
NH = 16
DH = 64
W = 512           # chunk width
C = L // W        # 8 chunks
J3 = 3 * W        # 1536 softmax width
NJ = J3 // 128    # 12 j-chunks
DMT = DM // 128   # 8 dm-chunks
NPAIR = NH // 2   # 8 head pairs
NLT = LS // 128   # 4 output row chunks


def build_nc():
    nc = bacc.Bacc(None, target_bir_lowering=False)

    qT = nc.dram_tensor("qT", [DM, LS], BF16, kind="ExternalInput")
    kvT = nc.dram_tensor("kvT", [DM, L], BF16, kind="ExternalInput")
    Wq = nc.dram_tensor("Wq", [DM, DM], BF16, kind="ExternalInput")
    Wkv = nc.dram_tensor("Wkv", [DM, 2 * DH], BF16, kind="ExternalInput")
    Wc = nc.dram_tensor("Wc", [DM, DM], BF16, kind="ExternalInput")
    out = nc.dram_tensor("out", [LS, DM], F32, kind="ExternalOutput")

    def chunked3d(dram, cols):
        # DRAM source AP delivering [128, DMT, cols]: slot d holds rows
        # 128d:128(d+1) of a [DM, cols] row-major tensor.
        base = dram[:, :]
        return bass.AP(tensor=base.tensor, offset=base.offset,
                       ap=[[cols, 128], [cols * 128, DMT], [1, cols]])

    with tile.TileContext(nc) as tc:
        with tc.tile_pool(name="weights", bufs=1) as wpool, \
             tc.tile_pool(name="small", bufs=1) as spool, \
             tc.tile_pool(name="qp", bufs=8) as qpool, \
             tc.tile_pool(name="stream", bufs=8) as stpool, \
             tc.tile_pool(name="kvsum", bufs=3) as kvspool, \
             tc.tile_pool(name="probs", bufs=2) as ppool, \
             tc.tile_pool(name="misc", bufs=2) as mpool, \
             tc.tile_pool(name="dram", bufs=1, space="DRAM") as dpool, \
             tc.tile_pool(name="psacc", bufs=4, space="PSUM") as psacc, \
             tc.tile_pool(name="psmm", bufs=2, space="PSUM") as psmm:

            # ---------- bulk loads: one DMA per tensor, 3 trigger rings --
            # ring plan (~2us fixed + bytes/436GB/s each, FIFO per ring):
            #   sync:   wkv, qT, kv0, kv3
            #   scalar: kv1, kv4, kv7, Wq
            #   gpsimd: kv2, kv5, kv6, Wc
            wkv_sb = wpool.tile([128, DMT, 2 * DH], BF16, tag="wkv")
            nc.sync.dma_start(out=wkv_sb, in_=chunked3d(Wkv, 2 * DH))
            qt_sb = wpool.tile([128, DMT, LS], BF16, tag="qt")
            nc.sync.dma_start(out=qt_sb, in_=chunked3d(qT, LS))
            st_sb = []
            for d in range(DMT):
                st = stpool.tile([128, L], BF16, tag="kvstream", name=f"st{d}")
                st_sb.append(st)
            kv_engs = (nc.sync, nc.scalar, nc.gpsimd, nc.sync,
                       nc.scalar, nc.gpsimd, nc.gpsimd, nc.scalar)
            for d in range(DMT):
                kv_engs[d].dma_start(out=st_sb[d],
                                     in_=kvT[128 * d:128 * (d + 1), :])
            wq_sb = wpool.tile([128, DMT, DM], BF16, tag="wq")
            nc.scalar.dma_start(out=wq_sb, in_=chunked3d(Wq, DM))
            wc_sb = wpool.tile([128, DMT, DM], BF16, tag="wc")
            nc.gpsimd.dma_start(out=wc_sb, in_=chunked3d(Wc, DM))

            # zero tile for PE warm-up (DVE memset: not gated on gpsimd)
            zt = spool.tile([128, 128], BF16, tag="zt")
            nc.vector.memset(zt, 0.0)
            ident = spool.tile([128, 128], BF16, tag="ident")
            make_identity(nc, ident)

            # ---------- PE warm-up during the DMA wait (HAM clock gate) --
            warm_ps = psacc.tile([128, W], F32, tag="acc", name="warm")
            for i in range(110):
                nc.tensor.matmul(warm_ps[:, 0:128], zt, zt,
                                 start=True, stop=True)
            # preload the exp activation table during startup
            exp_warm = spool.tile([1, 8], F32, tag="expwarm")
            nc.scalar.activation(exp_warm, zt[0:1, 0:8], AF.Exp, scale=1.0)

            # ---------- kv stream: chunk-sum tree + projections ----------
            # PSUM accumulators pack K rows 0:64, V rows 64:128.  The tree
            # alternates between DVE and GpSimd so neither is the gate.
            ps0 = psacc.tile([128, W], F32, tag="acc", name="ps0")
            ps7 = psacc.tile([128, W], F32, tag="acc", name="ps7")
            pss = psacc.tile([128, W], F32, tag="acc", name="pss")
            for d in range(DMT):
                st = st_sb[d]
                nc.tensor.matmul(ps0[0:DH, :], wkv_sb[:, d, 0:DH],
                                 st[:, 0:W], start=(d == 0), stop=(d == DMT - 1))
                nc.tensor.matmul(ps0[DH:128, :], wkv_sb[:, d, DH:2 * DH],
                                 st[:, 0:W], start=(d == 0), stop=(d == DMT - 1))
                nc.tensor.matmul(ps7[0:DH, :], wkv_sb[:, d, 0:DH],
                                 st[:, L - W:L], start=(d == 0), stop=(d == DMT - 1))
                nc.tensor.matmul(ps7[DH:128, :], wkv_sb[:, d, DH:2 * DH],
                                 st[:, L - W:L], start=(d == 0), stop=(d == DMT - 1))
                eng = nc.vector if d % 2 == 0 else nc.gpsimd
                eng.tensor_add(st[:, 0:2048], st[:, 0:2048], st[:, 2048:4096])
                eng.tensor_add(st[:, 0:1024], st[:, 0:1024], st[:, 1024:2048])
                ks = kvspool.tile([128, W], BF16, tag="kvsum")
                eng.tensor_add(ks, st[:, 0:512], st[:, 512:1024])
                nc.tensor.matmul(pss[0:DH, :], wkv_sb[:, d, 0:DH],
                                 ks, start=(d == 0), stop=(d == DMT - 1))
                nc.tensor.matmul(pss[DH:128, :], wkv_sb[:, d, DH:2 * DH],
                                 ks, start=(d == 0), stop=(d == DMT - 1))

            # ---------- evacuate K/V variants to SBUF (bf16) -------------
            kv0_sb = spool.tile([128, W], BF16, tag="kv0")  # K rows 0:64, V 64:128
            kv7_sb = spool.tile([128, W], BF16, tag="kv7")
            kvs_sb = spool.tile([128, W], BF16, tag="kvs")
            nc.vector.tensor_copy(kv0_sb, ps0)
            nc.vector.tensor_copy(kv7_sb, ps7)
            nc.vector.tensor_copy(kvs_sb, pss)

            # ---------- KbigT [128, 1536] = [prev | cur | next] ----------
            kbig = spool.tile([128, J3], BF16, tag="kbig")
            nc.vector.tensor_sub(kbig[0:DH, 0:W], kvs_sb[0:DH, :], kv7_sb[0:DH, :])
            nc.vector.tensor_copy(kbig[0:DH, W:2 * W], kvs_sb[0:DH, :])
            nc.vector.tensor_sub(kbig[0:DH, 2 * W:3 * W], kvs_sb[0:DH, :],
                                 kv0_sb[0:DH, :])
            nc.vector.tensor_copy(kbig[DH:2 * DH, :], kbig[0:DH, :])

            # ---------- Vbig [128, 12, 65(+pad)] -------------------------
            vbig = spool.tile([128, NJ, 68], BF16, tag="vbig")
            ones_sb = spool.tile([128, 1], BF16, tag="ones")
            nc.vector.memset(ones_sb, 1.0)
            for j in range(NJ):
                nc.vector.tensor_copy(vbig[:, j, DH:DH + 1], ones_sb)
            for yt in range(4):
                tps = psacc.tile([128, DH], BF16, tag="acc")
                tp0 = psacc.tile([128, DH], BF16, tag="acc")
                tp7 = psacc.tile([128, DH], BF16, tag="acc")
                sl = slice(128 * yt, 128 * (yt + 1))
                # V rows live at base partition 64; ident[64:128, 64:128]
                # is an identity block at the matching base.
                idq = ident[DH:128, DH:128]
                nc.tensor.transpose(tps, kvs_sb[DH:128, sl], idq)
                nc.tensor.transpose(tp0, kv0_sb[DH:128, sl], idq)
                nc.tensor.transpose(tp7, kv7_sb[DH:128, sl], idq)
                nc.vector.tensor_copy(vbig[:, 4 + yt, 0:DH], tps)
                nc.vector.tensor_sub(vbig[:, 0 + yt, 0:DH], vbig[:, 4 + yt, 0:DH], tp7)
                nc.vector.tensor_sub(vbig[:, 8 + yt, 0:DH], vbig[:, 4 + yt, 0:DH], tp0)

            # ---------- QP_T chunks (interleaved into the pair loop) -----
            qpt_sb = [None] * NPAIR

            def emit_qp_mm(ps, t, d):
                nc.tensor.matmul(ps, wq_sb[:, d, 128 * t:128 * (t + 1)],
                                 qt_sb[:, d, :],
                                 start=(d == 0), stop=(d == DMT - 1))

            def emit_qp(t):
                ps = psacc.tile([128, W], F32, tag="acc", name=f"qps{t}")
                for d in range(DMT):
                    emit_qp_mm(ps, t, d)
                sb = qpool.tile([128, LS], BF16, tag="qpt", name=f"qpt{t}")
                nc.vector.tensor_copy(sb, ps)
                qpt_sb[t] = sb

            # ---------- attention: QK -> exp(PSUM direct) -> PV ----------
            ctxu_sb = []  # per pair [128, 512]: rows 0:64 head 2t, 64:128 head 2t+1
            for t in range(NPAIR):
                ctxu_sb.append(qpool.tile([128, W], BF16, tag="ctxu",
                                          name=f"ctxu{t}"))
            rsc = dpool.tile([NPAIR, 2 * W], BF16, name="rsc")

            def normalize(t):
                # broadcast 1/denom (staged in DRAM) across partitions, then
                # scale ctxu in place.  Called one pair late so nothing here
                # is on the critical path.
                bc = mpool.tile([128, W], BF16, tag="bcast", name=f"bc{t}")
                src = bass.AP(tensor=rsc.tensor,
                              offset=rsc.offset + t * 2 * W,
                              ap=[[W, 2], [0, DH], [1, W]])
                nc.gpsimd.dma_start(out=bc, in_=src)
                nc.vector.tensor_mul(ctxu_sb[t], ctxu_sb[t], bc)

            emit_qp(0)
            emit_qp(1)
            for t in range(NPAIR):  # head pairs (2t, 2t+1)
                qpt = qpt_sb[t]
                ctxA = psacc.tile([128, W], F32, tag="acc", name=f"ctxA{t}")
                ctxB = psacc.tile([128, W], F32, tag="acc", name=f"ctxB{t}")
                qps = None
                if t + 2 < NPAIR:
                    qps = psacc.tile([128, W], F32, tag="acc",
                                     name=f"qps{t + 2}")
                for j in range(NJ):
                    qk = psmm.tile([128, 1024], F32, tag="mm", name=f"qk{t}_{j}")
                    nc.tensor.matmul(qk[:, 0:W],
                                     kbig[0:DH, 128 * j:128 * (j + 1)],
                                     qpt[0:DH, :], start=True, stop=True)
                    nc.tensor.matmul(qk[:, W:2 * W],
                                     kbig[DH:2 * DH, 128 * j:128 * (j + 1)],
                                     qpt[DH:128, :], start=True, stop=True)
                    pr = ppool.tile([128, 1024], BF16, tag="probs",
                                    name=f"pr{t}_{j}")
                    nc.scalar.activation(pr, qk, AF.Exp, scale=0.125)
                    nc.tensor.matmul(ctxA[0:DH + 1, :], vbig[:, j, 0:DH + 1],
                                     pr[:, 0:W],
                                     start=(j == 0), stop=(j == NJ - 1))
                    nc.tensor.matmul(ctxB[0:DH + 1, :], vbig[:, j, 0:DH + 1],
                                     pr[:, W:2 * W],
                                     start=(j == 0), stop=(j == NJ - 1))
                    # ride the next-next pair's QP_T matmuls in ACT's slack
                    if qps is not None and j < DMT:
                        emit_qp_mm(qps, t + 2, j)
                if qps is not None:
                    sb = qpool.tile([128, LS], BF16, tag="qpt",
                                    name=f"qpt{t + 2}")
                    nc.vector.tensor_copy(sb, qps)
                    qpt_sb[t + 2] = sb
                if t > 0:
                    normalize(t - 1)
                # context (cast bf16) + reciprocal of the denominator rows
                nc.vector.tensor_copy(ctxu_sb[t][0:DH, :], ctxA[0:DH, :])
                nc.vector.tensor_copy(ctxu_sb[t][DH:128, :], ctxB[0:DH, :])
                dtmp = mpool.tile([1, 2 * W], F32, tag="dtmp", name=f"dtmp{t}",
                                  bufs=2)
                nc.vector.tensor_copy(dtmp[:, 0:W], ctxA[DH:DH + 1, :])
                nc.vector.tensor_copy(dtmp[:, W:2 * W], ctxB[DH:DH + 1, :])
                rcf = mpool.tile([1, 2 * W], F32, tag="rcf", name=f"rcf{t}",
                                 bufs=2)
                nc.vector.reciprocal(rcf, dtmp)
                rc16 = mpool.tile([1, 2 * W], BF16, tag="rc16", name=f"rc16{t}",
                                  bufs=2)
                nc.vector.tensor_copy(rc16, rcf)
                nc.sync.dma_start(out=rsc[t:t + 1, :], in_=rc16)

            # ---------- out = ctx @ Wc ----------
            # he chunks 0..6 of the first two row blocks run while the last
            # pair's normalization chain completes; outputs evacuate on the
            # otherwise-idle scalar engine.
            def emit_wc(lt, he_list):
                ps = wc_ps[lt]
                for half in range(2):
                    for he in he_list:
                        nc.tensor.matmul(
                            ps[:, 512 * half:512 * (half + 1)],
                            ctxu_sb[he][:, 128 * lt:128 * (lt + 1)],
                            wc_sb[:, he, 512 * half:512 * (half + 1)],
                            start=(he == 0), stop=(he == DMT - 1))

            def emit_out(lt):
                ob = mpool.tile([128, DM], F32, tag="outsb", bufs=2)
                nc.scalar.activation(ob, wc_ps[lt], AF.Copy)
                nc.sync.dma_start(out=out[128 * lt:128 * (lt + 1), :], in_=ob)

            wc_ps = {}
            for lt in (0, 1):
                wc_ps[lt] = psmm.tile([128, 1024], F32, tag="mm", name=f"wcp{lt}")
                emit_wc(lt, range(7))
            normalize(NPAIR - 1)
            for lt in (0, 1):
                emit_wc(lt, [7])
                emit_out(lt)
            for lt in (2, 3):
                wc_ps[lt] = psmm.tile([128, 1024], F32, tag="mm", name=f"wcp{lt}")
                emit_wc(lt, range(8))
                emit_out(lt)

    nc.compile()
    return nc


_NC = None


def _get_nc():
    global _NC
    if _NC is None:
        _NC = build_nc()
    return _NC


def prep_in_maps(q, kv, Wq, Wkv, Wc):
    """Host-side input prep: transpose, cast to bf16, shard queries."""
    bf16 = ml_dtypes.bfloat16
    qT_full = np.ascontiguousarray(np.asarray(q, dtype=np.float32)[0].T
                                   ).astype(bf16)
    kvT = np.ascontiguousarray(np.asarray(kv, dtype=np.float32)[0].T
                               ).astype(bf16)
    Wq = np.ascontiguousarray(np.asarray(Wq, dtype=np.float32)).astype(bf16)
    Wkv = np.ascontiguousarray(np.asarray(Wkv, dtype=np.float32)).astype(bf16)
    Wc = np.ascontiguousarray(np.asarray(Wc, dtype=np.float32)).astype(bf16)
    in_maps = []
    for i in range(N_CORES):
        in_maps.append({
            "qT": np.ascontiguousarray(qT_full[:, LS * i:LS * (i + 1)]),
            "kvT": kvT,
            "Wq": Wq,
            "Wkv": Wkv,
            "Wc": Wc,
        })
    return in_maps


def kernel(q, kv, Wq, Wkv, Wc, w):
    assert int(w) == W
    q = np.asarray(q, dtype=np.float32)
    B = q.shape[0]
    assert B == 1 and q.shape[1] == L and q.shape[2] == DM

    in_maps = prep_in_maps(q, kv, Wq, Wkv, Wc)
    nc = _get_nc()
    res = run_bass_kernel_spmd(nc, in_maps, list(range(N_CORES)))
    out = np.concatenate([res.results[i]["out"] for i in range(N_CORES)], axis=0)
    return out.reshape(1, L, DM).astype(np.float32)


# revision 24
# speedup vs baseline: 1.3202x; 1.3202x over previous
"""Trainium2 Bass kernel for LocalXLAttention (chunk-summed variant).

Math: the reference einsum sums over the chunk index z, so every query
attends to the same three [w, dh] K/V matrices built from chunk sums:
  K_prev = S_k - k_chunk[C-1], K_cur = S_k, K_next = S_k - k_chunk[0]
(and identically for V), where S_k = sum_c k_chunk[c].  The computation
collapses to, per sequence position l and head h:
  attn[l,h,:]  = qp[l,h,:] @ KbigT          (KbigT: [dh, 3w])
  probs        = softmax(attn, axis=-1)
  ctx[l,h,:]   = probs[l,h,:] @ Vbig        (Vbig:  [3w, dh])
  out          = ctx.reshape(L, dm) @ Wc

Sharding: L=4096 is split 512 rows per core across 8 NeuronCores
(data-parallel over the sequence; no collectives).  Each core redundantly
computes the tiny chunk-summed K/V from the full kv input.

All matmul operands are bf16 (fp32 PSUM accumulation).  exp runs on the
scalar engine directly from PSUM (its 1 elem/cycle/lane rate is the
kernel's floor); the attention loop is organized so ACT never waits:
even PSUM ring parity across pairs, QP_T chunks and the deferred softmax
normalization ride in the PE/DVE slack.  Inputs load as one large DMA
per tensor (the ~2us fixed cost per dma_start dominates small
transfers), spread across all five trigger-engine rings.
"""

import sys
for _p in ('/opt/pypackages', '/opt/trn_rl_repo'):
    if _p not in sys.path:
        sys.path.insert(0, _p)

import numpy as np
import ml_dtypes

import concourse.bass as bass
import concourse.bacc as bacc
import concourse.tile as tile
from concourse import mybir
from concourse.bass_utils import run_bass_kernel_spmd
from concourse.masks import make_identity

F32 = mybir.dt.float32
BF16 = mybir.dt.bfloat16
AF = mybir.ActivationFunctionType

N_CORES = 8
L = 4096          # full sequence
LS = L // N_CORES # 512 rows per core
DM = 1024
NH = 16
DH = 64
W = 512           # chunk width
C = L // W        # 8 chunks
J3 = 3 * W        # 1536 softmax width
NJ = J3 // 128    # 12 j-chunks
DMT = DM // 128   # 8 dm-chunks
NPAIR = NH // 2   # 8 head pairs
NLT = LS // 128   # 4 output row chunks


def build_nc():
    nc = bacc.Bacc(None, target_bir_lowering=False)

    qT = nc.dram_tensor("qT", [DM, LS], BF16, kind="ExternalInput")
    kvT = nc.dram_tensor("kvT", [DM, L], BF16, kind="ExternalInput")
    Wq = nc.dram_tensor("Wq", [DM, DM], BF16, kind="ExternalInput")
    Wkv = nc.dram_tensor("Wkv", [DM, 2 * DH], BF16, kind="ExternalInput")
    Wc = nc.dram_tensor("Wc", [DM, DM], BF16, kind="ExternalInput")
    out = nc.dram_tensor("out", [LS, DM], F32, kind="ExternalOutput")

    def chunked3d(dram, cols):
        # DRAM source AP delivering [128, DMT, cols]: slot d holds rows
        # 128d:128(d+1) of a [DM, cols] row-major tensor.
        base = dram[:, :]
        return bass.AP(tensor=base.tensor, offset=base.offset,
                       ap=[[cols, 128], [cols * 128, DMT], [1, cols]])

    with tile.TileContext(nc) as tc:
        with tc.tile_pool(name="weights", bufs=1) as wpool, \
             tc.tile_pool(name="small", bufs=1) as spool, \
             tc.tile_pool(name="qp", bufs=8) as qpool, \
             tc.tile_pool(name="stream", bufs=8) as stpool, \
             tc.tile_pool(name="kvsum", bufs=3) as kvspool, \
             tc.tile_pool(name="probs", bufs=2) as ppool, \
             tc.tile_pool(name="misc", bufs=2) as mpool, \
             tc.tile_pool(name="dram", bufs=1, space="DRAM") as dpool, \
             tc.tile_pool(name="psacc", bufs=4, space="PSUM") as psacc, \
             tc.tile_pool(name="psmm", bufs=2, space="PSUM") as psmm:

            # ---------- bulk loads: ~256KB pieces across 3 trigger rings -
            # Each dma_start drains on one queue at ~23GB/s; aggregate
            # bandwidth comes from many concurrent DMAs.  kv pieces go
            # first in d-order (the chunk-sum tree consumes them in
            # order), then qT/Wq, Wc last.
            ENGS = (nc.sync, nc.scalar, nc.gpsimd)
            wkv_sb = wpool.tile([128, DMT, 2 * DH], BF16, tag="wkv")
            nc.sync.dma_start(out=wkv_sb, in_=chunked3d(Wkv, 2 * DH))
            st_sb = []
            for d in range(DMT):
                st = stpool.tile([128, L], BF16, tag="kvstream", name=f"st{d}")
                st_sb.append(st)
            n = 1
            for d in range(DMT):
                for q in range(4):
                    ENGS[n % 3].dma_start(
                        out=st_sb[d][:, 1024 * q:1024 * (q + 1)],
                        in_=kvT[128 * d:128 * (d + 1), 1024 * q:1024 * (q + 1)])
                    n += 1
            qt_sb = wpool.tile([128, DMT, LS], BF16, tag="qt")
            qt_base = qT[:, :]
            for h in range(4):
                src = bass.AP(tensor=qt_base.tensor,
                              offset=qt_base.offset + 2 * h * 128 * LS,
                              ap=[[LS, 128], [LS * 128, 2], [1, LS]])
                ENGS[n % 3].dma_start(out=qt_sb[:, 2 * h:2 * h + 2, :], in_=src)
                n += 1
            wq_sb = wpool.tile([128, DMT, DM], BF16, tag="wq")
            for d in range(DMT):
                ENGS[n % 3].dma_start(out=wq_sb[:, d, :],
                                      in_=Wq[128 * d:128 * (d + 1), :])
                n += 1
            wc_sb = wpool.tile([128, DMT, DM], BF16, tag="wc")
            for d in range(DMT):
                ENGS[n % 3].dma_start(out=wc_sb[:, d, :],
                                      in_=Wc[128 * d:128 * (d + 1), :])
                n += 1

            # zero tile for PE warm-up (DVE memset: not gated on gpsimd)
            zt = spool.tile([128, 128], BF16, tag="zt")
            nc.vector.memset(zt, 0.0)
            ident = spool.tile([128, 128], BF16, tag="ident")
            make_identity(nc, ident)

            # ---------- PE warm-up during the DMA wait (HAM clock gate) --
            warm_ps = psacc.tile([128, W], F32, tag="acc", name="warm")
            for i in range(110):
                nc.tensor.matmul(warm_ps[:, 0:128], zt, zt,
                                 start=True, stop=True)
            # preload the exp activation table during startup
            exp_warm = spool.tile([1, 8], F32, tag="expwarm")
            nc.scalar.activation(exp_warm, zt[0:1, 0:8], AF.Exp, scale=1.0)

            # ---------- kv stream: chunk-sum tree + projections ----------
            # PSUM accumulators pack K rows 0:64, V rows 64:128.  The tree
            # alternates between DVE and GpSimd so neither is the gate.
            ps0 = psacc.tile([128, W], F32, tag="acc", name="ps0")
            ps7 = psacc.tile([128, W], F32, tag="acc", name="ps7")
            pss = psacc.tile([128, W], F32, tag="acc", name="pss")
            for d in range(DMT):
                st = st_sb[d]
                nc.tensor.matmul(ps0[0:DH, :], wkv_sb[:, d, 0:DH],
                                 st[:, 0:W], start=(d == 0), stop=(d == DMT - 1))
                nc.tensor.matmul(ps0[DH:128, :], wkv_sb[:, d, DH:2 * DH],
                                 st[:, 0:W], start=(d == 0), stop=(d == DMT - 1))
                nc.tensor.matmul(ps7[0:DH, :], wkv_sb[:, d, 0:DH],
                                 st[:, L - W:L], start=(d == 0), stop=(d == DMT - 1))
                nc.tensor.matmul(ps7[DH:128, :], wkv_sb[:, d, DH:2 * DH],
                                 st[:, L - W:L], start=(d == 0), stop=(d == DMT - 1))
                nc.vector.tensor_add(st[:, 0:2048], st[:, 0:2048], st[:, 2048:4096])
                nc.vector.tensor_add(st[:, 0:1024], st[:, 0:1024], st[:, 1024:2048])
                ks = kvspool.tile([128, W], BF16, tag="kvsum")
                nc.vector.tensor_add(ks, st[:, 0:512], st[:, 512:1024])
                nc.tensor.matmul(pss[0:DH, :], wkv_sb[:, d, 0:DH],
                                 ks, start=(d == 0), stop=(d == DMT - 1))
                nc.tensor.matmul(pss[DH:128, :], wkv_sb[:, d, DH:2 * DH],
                                 ks, start=(d == 0), stop=(d == DMT - 1))

            # ---------- evacuate K/V variants to SBUF (bf16) -------------
            kv0_sb = spool.tile([128, W], BF16, tag="kv0")  # K rows 0:64, V 64:128
            kv7_sb = spool.tile([128, W], BF16, tag="kv7")
            kvs_sb = spool.tile([128, W], BF16, tag="kvs")
            nc.vector.tensor_copy(kv0_sb, ps0)
            nc.vector.tensor_copy(kv7_sb, ps7)
            nc.vector.tensor_copy(kvs_sb, pss)

            # ---------- KbigT [128, 1536] = [prev | cur | next] ----------
            kbig = spool.tile([128, J3], BF16, tag="kbig")
            nc.vector.tensor_sub(kbig[0:DH, 0:W], kvs_sb[0:DH, :], kv7_sb[0:DH, :])
            nc.vector.tensor_copy(kbig[0:DH, W:2 * W], kvs_sb[0:DH, :])
            nc.vector.tensor_sub(kbig[0:DH, 2 * W:3 * W], kvs_sb[0:DH, :],
                                 kv0_sb[0:DH, :])
            nc.vector.tensor_copy(kbig[DH:2 * DH, :], kbig[0:DH, :])

            # ---------- Vbig [128, 12, 65(+pad)] -------------------------
            vbig = spool.tile([128, NJ, 68], BF16, tag="vbig")
            ones_sb = spool.tile([128, 1], BF16, tag="ones")
            nc.vector.memset(ones_sb, 1.0)
            for j in range(NJ):
                nc.vector.tensor_copy(vbig[:, j, DH:DH + 1], ones_sb)
            for yt in range(4):
                tps = psacc.tile([128, DH], BF16, tag="acc")
                tp0 = psacc.tile([128, DH], BF16, tag="acc")
                tp7 = psacc.tile([128, DH], BF16, tag="acc")
                sl = slice(128 * yt, 128 * (yt + 1))
                # V rows live at base partition 64; ident[64:128, 64:128]
                # is an identity block at the matching base.
                idq = ident[DH:128, DH:128]
                nc.tensor.transpose(tps, kvs_sb[DH:128, sl], idq)
                nc.tensor.transpose(tp0, kv0_sb[DH:128, sl], idq)
                nc.tensor.transpose(tp7, kv7_sb[DH:128, sl], idq)
                nc.vector.tensor_copy(vbig[:, 4 + yt, 0:DH], tps)
                nc.vector.tensor_sub(vbig[:, 0 + yt, 0:DH], vbig[:, 4 + yt, 0:DH], tp7)
                nc.vector.tensor_sub(vbig[:, 8 + yt, 0:DH], vbig[:, 4 + yt, 0:DH], tp0)

            # ---------- QP_T chunks (interleaved into the pair loop) -----
            qpt_sb = [None] * NPAIR

            def emit_qp_mm(ps, t, d):
                nc.tensor.matmul(ps, wq_sb[:, d, 128 * t:128 * (t + 1)],
                                 qt_sb[:, d, :],
                                 start=(d == 0), stop=(d == DMT - 1))

            def emit_qp(t):
                ps = psacc.tile([128, W], F32, tag="acc", name=f"qps{t}")
                for d in range(DMT):
                    emit_qp_mm(ps, t, d)
                sb = qpool.tile([128, LS], BF16, tag="qpt", name=f"qpt{t}")
                nc.vector.tensor_copy(sb, ps)
                qpt_sb[t] = sb

            # ---------- attention: QK -> exp(PSUM direct) -> PV ----------
            ctxu_sb = []  # per pair [128, 512]: rows 0:64 head 2t, 64:128 head 2t+1
            for t in range(NPAIR):
                ctxu_sb.append(qpool.tile([128, W], BF16, tag="ctxu",
                                          name=f"ctxu{t}"))
            rsc = dpool.tile([NPAIR, 2 * W], BF16, name="rsc")
            dnd = dpool.tile([NPAIR, 2 * W], F32, name="dnd")

            def normalize(t):
                # broadcast 1/denom (staged in DRAM) across partitions, then
                # scale ctxu in place.  Called one pair late so nothing here
                # is on the critical path.
                bc = mpool.tile([128, W], BF16, tag="bcast", name=f"bc{t}")
                src = bass.AP(tensor=rsc.tensor,
                              offset=rsc.offset + t * 2 * W,
                              ap=[[W, 2], [0, DH], [1, W]])
                nc.gpsimd.dma_start(out=bc, in_=src)
                nc.vector.tensor_mul(ctxu_sb[t], ctxu_sb[t], bc)

            emit_qp(0)
            emit_qp(1)
            for t in range(NPAIR):  # head pairs (2t, 2t+1)
                qpt = qpt_sb[t]
                ctxA = psacc.tile([128, W], F32, tag="acc", name=f"ctxA{t}")
                ctxB = psacc.tile([128, W], F32, tag="acc", name=f"ctxB{t}")
                qps = None
                if t + 2 < NPAIR:
                    qps = psacc.tile([128, W], F32, tag="acc",
                                     name=f"qps{t + 2}")
                for j in range(NJ):
                    qk = psmm.tile([128, 1024], F32, tag="mm", name=f"qk{t}_{j}")
                    nc.tensor.matmul(qk[:, 0:W],
                                     kbig[0:DH, 128 * j:128 * (j + 1)],
                                     qpt[0:DH, :], start=True, stop=True)
                    nc.tensor.matmul(qk[:, W:2 * W],
                                     kbig[DH:2 * DH, 128 * j:128 * (j + 1)],
                                     qpt[DH:128, :], start=True, stop=True)
                    pr = ppool.tile([128, 1024], BF16, tag="probs",
                                    name=f"pr{t}_{j}")
                    nc.scalar.activation(pr, qk, AF.Exp, scale=0.125)
                    nc.tensor.matmul(ctxA[0:DH + 1, :], vbig[:, j, 0:DH + 1],
                                     pr[:, 0:W],
                                     start=(j == 0), stop=(j == NJ - 1))
                    nc.tensor.matmul(ctxB[0:DH + 1, :], vbig[:, j, 0:DH + 1],
                                     pr[:, W:2 * W],
                                     start=(j == 0), stop=(j == NJ - 1))
                    # ride the next-next pair's QP_T matmuls in ACT's slack
                    if qps is not None and j < DMT:
                        emit_qp_mm(qps, t + 2, j)
                if qps is not None:
                    sb = qpool.tile([128, LS], BF16, tag="qpt",
                                    name=f"qpt{t + 2}")
                    nc.vector.tensor_copy(sb, qps)
                    qpt_sb[t + 2] = sb
                if t > 0:
                    normalize(t - 1)
                # context (cast bf16) + reciprocal of the denominator rows
                nc.vector.tensor_copy(ctxu_sb[t][0:DH, :], ctxA[0:DH, :])
                nc.vector.tensor_copy(ctxu_sb[t][DH:128, :], ctxB[0:DH, :])
                # denominators: bounce through DRAM to respread [1, 1024]
                # over 32 partitions (single-partition DVE ops are ~6us).
                dtmp = mpool.tile([1, 2 * W], F32, tag="dtmp", name=f"dtmp{t}",
                                  bufs=2)
                nc.vector.tensor_copy(dtmp[:, 0:W], ctxA[DH:DH + 1, :])
                nc.vector.tensor_copy(dtmp[:, W:2 * W], ctxB[DH:DH + 1, :])
                nc.sync.dma_start(out=dnd[t:t + 1, :], in_=dtmp)
                sq = mpool.tile([32, 32], F32, tag="sq", name=f"sq{t}", bufs=2)
                srcd = bass.AP(tensor=dnd.tensor, offset=dnd.offset + t * 2 * W,
                               ap=[[32, 32], [1, 32]])
                nc.gpsimd.dma_start(out=sq, in_=srcd)
                rq = mpool.tile([32, 32], F32, tag="rq", name=f"rq{t}", bufs=2)
                nc.vector.reciprocal(rq, sq)
                rq16 = mpool.tile([32, 32], BF16, tag="rq16", name=f"rq16{t}",
                                  bufs=2)
                nc.vector.tensor_copy(rq16, rq)
                nc.sync.dma_start(out=rsc[t:t + 1, :], in_=rq16)

            # ---------- out = ctx @ Wc ----------
            # he chunks 0..6 of the first two row blocks run while the last
            # pair's normalization chain completes; outputs evacuate on the
            # otherwise-idle scalar engine.
            def emit_wc(lt, he_list):
                ps = wc_ps[lt]
                for half in range(2):
                    for he in he_list:
                        nc.tensor.matmul(
                            ps[:, 512 * half:512 * (half + 1)],
                            ctxu_sb[he][:, 128 * lt:128 * (lt + 1)],
                            wc_sb[:, he, 512 * half:512 * (half + 1)],
                            start=(he == 0), stop=(he == DMT - 1))

            def emit_out(lt):
                ob = mpool.tile([128, DM], F32, tag="outsb", bufs=2)
                nc.scalar.activation(ob, wc_ps[lt], AF.Copy)
                nc.sync.dma_start(out=out[128 * lt:128 * (lt + 1), :], in_=ob)

            wc_ps = {}
            for lt in (0, 1):
                wc_ps[lt] = psmm.tile([128, 1024], F32, tag="mm", name=f"wcp{lt}")
                emit_wc(lt, range(7))
            normalize(NPAIR - 1)
            for lt in (0, 1):
                emit_wc(lt, [7])
                emit_out(lt)
            for lt in (2, 3):
                wc_ps[lt] = psmm.tile([128, 1024], F32, tag="mm", name=f"wcp{lt}")
                emit_wc(lt, range(8))
                emit_out(lt)

    nc.compile()
    return nc


_NC = None


def _get_nc():
    global _NC
    if _NC is None:
        _NC = build_nc()
    return _NC


def prep_in_maps(q, kv, Wq, Wkv, Wc):
    """Host-side input prep: transpose, cast to bf16, shard queries."""
    bf16 = ml_dtypes.bfloat16
    qT_full = np.ascontiguousarray(np.asarray(q, dtype=np.float32)[0].T
                                   ).astype(bf16)
    kvT = np.ascontiguousarray(np.asarray(kv, dtype=np.float32)[0].T
                               ).astype(bf16)
    Wq = np.ascontiguousarray(np.asarray(Wq, dtype=np.float32)).astype(bf16)
    Wkv = np.ascontiguousarray(np.asarray(Wkv, dtype=np.float32)).astype(bf16)
    Wc = np.ascontiguousarray(np.asarray(Wc, dtype=np.float32)).astype(bf16)
    in_maps = []
    for i in range(N_CORES):
        in_maps.append({
            "qT": np.ascontiguousarray(qT_full[:, LS * i:LS * (i + 1)]),
            "kvT": kvT,
            "Wq": Wq,
            "Wkv": Wkv,
            "Wc": Wc,
        })
    return in_maps


def kernel(q, kv, Wq, Wkv, Wc, w):
    assert int(w) == W
    q = np.asarray(q, dtype=np.float32)
    B = q.shape[0]
    assert B == 1 and q.shape[1] == L and q.shape[2] == DM

    in_maps = prep_in_maps(q, kv, Wq, Wkv, Wc)
    nc = _get_nc()
    res = run_bass_kernel_spmd(nc, in_maps, list(range(N_CORES)))
    out = np.concatenate([res.results[i]["out"] for i in range(N_CORES)], axis=0)
    return out.reshape(1, L, DM).astype(np.float32)


# revision 28
# speedup vs baseline: 1.3586x; 1.0291x over previous
"""Trainium2 Bass kernel for LocalXLAttention (chunk-summed variant).

Math: the reference einsum sums over the chunk index z, so every query
attends to the same three [w, dh] K/V matrices built from chunk sums:
  K_prev = S_k - k_chunk[C-1], K_cur = S_k, K_next = S_k - k_chunk[0]
(and identically for V), where S_k = sum_c k_chunk[c].  The computation
collapses to, per sequence position l and head h:
  attn[l,h,:]  = qp[l,h,:] @ KbigT          (KbigT: [dh, 3w])
  probs        = softmax(attn, axis=-1)
  ctx[l,h,:]   = probs[l,h,:] @ Vbig        (Vbig:  [3w, dh])
  out          = ctx.reshape(L, dm) @ Wc

Sharding: L=4096 is split 512 rows per core across 8 NeuronCores
(data-parallel over the sequence; no collectives).  Each core redundantly
computes the tiny chunk-summed K/V from the full kv input.

All matmul operands are bf16 (fp32 PSUM accumulation).  exp runs on the
scalar engine directly from PSUM (its 1 elem/cycle/lane rate is the
kernel's floor); the attention loop is organized so ACT never waits:
even PSUM ring parity across pairs, QP_T chunks and the deferred softmax
normalization ride in the PE/DVE slack.  Inputs load as one large DMA
per tensor (the ~2us fixed cost per dma_start dominates small
transfers), spread across all five trigger-engine rings.
"""

import sys
for _p in ('/opt/pypackages', '/opt/trn_rl_repo'):
    if _p not in sys.path:
        sys.path.insert(0, _p)

import numpy as np
import ml_dtypes

import concourse.bass as bass
import concourse.bacc as bacc
import concourse.tile as tile
from concourse import mybir
from concourse.bass_utils import run_bass_kernel_spmd
from concourse.masks import make_identity

F32 = mybir.dt.float32
BF16 = mybir.dt.bfloat16
AF = mybir.ActivationFunctionType

N_CORES = 8
L = 4096          # full sequence
LS = L // N_CORES # 512 rows per core
DM = 1024
NH = 16
DH = 64
W = 512           # chunk width
C = L // W        # 8 chunks
J3 = 3 * W        # 1536 softmax width
NJ = J3 // 128    # 12 j-chunks
DMT = DM // 128   # 8 dm-chunks
NPAIR = NH // 2   # 8 head pairs
NLT = LS // 128   # 4 output row chunks


def build_nc():
    nc = bacc.Bacc(None, target_bir_lowering=False)

    qT = nc.dram_tensor("qT", [DM, LS], BF16, kind="ExternalInput")
    kvT = nc.dram_tensor("kvT", [DM, L], BF16, kind="ExternalInput")
    Wq = nc.dram_tensor("Wq", [DM, DM], BF16, kind="ExternalInput")
    Wkv = nc.dram_tensor("Wkv", [DM, 2 * DH], BF16, kind="ExternalInput")
    Wc = nc.dram_tensor("Wc", [DM, DM], BF16, kind="ExternalInput")
    out = nc.dram_tensor("out", [LS, DM], F32, kind="ExternalOutput")

    def chunked3d(dram, cols):
        # DRAM source AP delivering [128, DMT, cols]: slot d holds rows
        # 128d:128(d+1) of a [DM, cols] row-major tensor.
        base = dram[:, :]
        return bass.AP(tensor=base.tensor, offset=base.offset,
                       ap=[[cols, 128], [cols * 128, DMT], [1, cols]])

    with tile.TileContext(nc) as tc:
        with tc.tile_pool(name="weights", bufs=1) as wpool, \
             tc.tile_pool(name="small", bufs=1) as spool, \
             tc.tile_pool(name="qp", bufs=8) as qpool, \
             tc.tile_pool(name="stream", bufs=8) as stpool, \
             tc.tile_pool(name="kvsum", bufs=3) as kvspool, \
             tc.tile_pool(name="probs", bufs=2) as ppool, \
             tc.tile_pool(name="misc", bufs=2) as mpool, \
             tc.tile_pool(name="dram", bufs=1, space="DRAM") as dpool, \
             tc.tile_pool(name="psacc", bufs=4, space="PSUM") as psacc, \
             tc.tile_pool(name="psmm", bufs=2, space="PSUM") as psmm:

            # ---------- bulk loads: ~256KB pieces across 3 trigger rings -
            # Each dma_start drains on one queue at ~23GB/s; aggregate
            # bandwidth comes from many concurrent DMAs.  kv pieces go
            # first in d-order (the chunk-sum tree consumes them in
            # order), then qT/Wq, Wc last.
            ENGS = (nc.sync, nc.scalar, nc.gpsimd)
            wkv_sb = wpool.tile([128, DMT, 2 * DH], BF16, tag="wkv")
            nc.sync.dma_start(out=wkv_sb, in_=chunked3d(Wkv, 2 * DH))
            st_sb = []
            for d in range(DMT):
                st = stpool.tile([128, L], BF16, tag="kvstream", name=f"st{d}")
                st_sb.append(st)
            n = 1
            for d in range(DMT):
                for q in range(4):
                    ENGS[n % 3].dma_start(
                        out=st_sb[d][:, 1024 * q:1024 * (q + 1)],
                        in_=kvT[128 * d:128 * (d + 1), 1024 * q:1024 * (q + 1)])
                    n += 1
            qt_sb = wpool.tile([128, DMT, LS], BF16, tag="qt")
            qt_base = qT[:, :]
            for h in range(4):
                src = bass.AP(tensor=qt_base.tensor,
                              offset=qt_base.offset + 2 * h * 128 * LS,
                              ap=[[LS, 128], [LS * 128, 2], [1, LS]])
                ENGS[n % 3].dma_start(out=qt_sb[:, 2 * h:2 * h + 2, :], in_=src)
                n += 1
            wq_sb = wpool.tile([128, DMT, DM], BF16, tag="wq")
            for d in range(DMT):
                ENGS[n % 3].dma_start(out=wq_sb[:, d, :],
                                      in_=Wq[128 * d:128 * (d + 1), :])
                n += 1
            wc_sb = wpool.tile([128, DMT, DM], BF16, tag="wc")
            for d in range(DMT):
                ENGS[n % 3].dma_start(out=wc_sb[:, d, :],
                                      in_=Wc[128 * d:128 * (d + 1), :])
                n += 1

            # zero tile for PE warm-up (DVE memset: not gated on gpsimd)
            zt = spool.tile([128, 128], BF16, tag="zt")
            nc.vector.memset(zt, 0.0)
            ident = spool.tile([128, 128], BF16, tag="ident")
            make_identity(nc, ident)

            # ---------- PE warm-up during the DMA wait (HAM clock gate) --
            warm_ps = psacc.tile([128, W], F32, tag="acc", name="warm")
            for i in range(110):
                nc.tensor.matmul(warm_ps[:, 0:128], zt, zt,
                                 start=True, stop=True)
            # preload the exp activation table during startup
            exp_warm = spool.tile([1, 8], F32, tag="expwarm")
            nc.scalar.activation(exp_warm, zt[0:1, 0:8], AF.Exp, scale=1.0)

            # ---------- kv stream: chunk-sum tree + projections ----------
            # PSUM accumulators pack K rows 0:64, V rows 64:128.  The tree
            # alternates between DVE and GpSimd so neither is the gate.
            ps0 = psacc.tile([128, W], F32, tag="acc", name="ps0")
            ps7 = psacc.tile([128, W], F32, tag="acc", name="ps7")
            pss = psacc.tile([128, W], F32, tag="acc", name="pss")
            for d in range(DMT):
                st = st_sb[d]
                nc.tensor.matmul(ps0[0:DH, :], wkv_sb[:, d, 0:DH],
                                 st[:, 0:W], start=(d == 0), stop=(d == DMT - 1))
                nc.tensor.matmul(ps0[DH:128, :], wkv_sb[:, d, DH:2 * DH],
                                 st[:, 0:W], start=(d == 0), stop=(d == DMT - 1))
                nc.tensor.matmul(ps7[0:DH, :], wkv_sb[:, d, 0:DH],
                                 st[:, L - W:L], start=(d == 0), stop=(d == DMT - 1))
                nc.tensor.matmul(ps7[DH:128, :], wkv_sb[:, d, DH:2 * DH],
                                 st[:, L - W:L], start=(d == 0), stop=(d == DMT - 1))
                nc.vector.tensor_add(st[:, 0:2048], st[:, 0:2048], st[:, 2048:4096])
                nc.vector.tensor_add(st[:, 0:1024], st[:, 0:1024], st[:, 1024:2048])
                ks = kvspool.tile([128, W], BF16, tag="kvsum")
                nc.vector.tensor_add(ks, st[:, 0:512], st[:, 512:1024])
                nc.tensor.matmul(pss[0:DH, :], wkv_sb[:, d, 0:DH],
                                 ks, start=(d == 0), stop=(d == DMT - 1))
                nc.tensor.matmul(pss[DH:128, :], wkv_sb[:, d, DH:2 * DH],
                                 ks, start=(d == 0), stop=(d == DMT - 1))

            # ---------- evacuate K/V variants to SBUF (bf16) -------------
            kv0_sb = spool.tile([128, W], BF16, tag="kv0")  # K rows 0:64, V 64:128
            kv7_sb = spool.tile([128, W], BF16, tag="kv7")
            kvs_sb = spool.tile([128, W], BF16, tag="kvs")
            nc.vector.tensor_copy(kv0_sb, ps0)
            nc.vector.tensor_copy(kv7_sb, ps7)
            nc.vector.tensor_copy(kvs_sb, pss)

            # ---------- KbigT [128, 1536] = [prev | cur | next] ----------
            kbig = spool.tile([128, J3], BF16, tag="kbig")
            nc.vector.tensor_sub(kbig[0:DH, 0:W], kvs_sb[0:DH, :], kv7_sb[0:DH, :])
            nc.vector.tensor_copy(kbig[0:DH, W:2 * W], kvs_sb[0:DH, :])
            nc.vector.tensor_sub(kbig[0:DH, 2 * W:3 * W], kvs_sb[0:DH, :],
                                 kv0_sb[0:DH, :])
            nc.vector.tensor_copy(kbig[DH:2 * DH, :], kbig[0:DH, :])

            # ---------- Vbig [128, 12, 65(+pad)] -------------------------
            vbig = spool.tile([128, NJ, 68], BF16, tag="vbig")
            ones_sb = spool.tile([128, 1], BF16, tag="ones")
            nc.vector.memset(ones_sb, 1.0)
            for j in range(NJ):
                nc.vector.tensor_copy(vbig[:, j, DH:DH + 1], ones_sb)
            for yt in range(4):
                tps = psacc.tile([128, DH], BF16, tag="acc")
                tp0 = psacc.tile([128, DH], BF16, tag="acc")
                tp7 = psacc.tile([128, DH], BF16, tag="acc")
                sl = slice(128 * yt, 128 * (yt + 1))
                # V rows live at base partition 64; ident[64:128, 64:128]
                # is an identity block at the matching base.
                idq = ident[DH:128, DH:128]
                nc.tensor.transpose(tps, kvs_sb[DH:128, sl], idq)
                nc.tensor.transpose(tp0, kv0_sb[DH:128, sl], idq)
                nc.tensor.transpose(tp7, kv7_sb[DH:128, sl], idq)
                nc.vector.tensor_copy(vbig[:, 4 + yt, 0:DH], tps)
                nc.vector.tensor_sub(vbig[:, 0 + yt, 0:DH], vbig[:, 4 + yt, 0:DH], tp7)
                nc.vector.tensor_sub(vbig[:, 8 + yt, 0:DH], vbig[:, 4 + yt, 0:DH], tp0)

            # ---------- QP_T chunks (interleaved into the pair loop) -----
            qpt_sb = [None] * NPAIR

            def emit_qp_mm(ps, t, d):
                nc.tensor.matmul(ps, wq_sb[:, d, 128 * t:128 * (t + 1)],
                                 qt_sb[:, d, :],
                                 start=(d == 0), stop=(d == DMT - 1))

            def emit_qp(t):
                ps = psacc.tile([128, W], F32, tag="acc", name=f"qps{t}")
                for d in range(DMT):
                    emit_qp_mm(ps, t, d)
                sb = qpool.tile([128, LS], BF16, tag="qpt", name=f"qpt{t}")
                nc.vector.tensor_copy(sb, ps)
                qpt_sb[t] = sb

            # ---------- attention: QK -> exp(PSUM direct) -> PV ----------
            ctxu_sb = []  # per pair [128, 512]: rows 0:64 head 2t, 64:128 head 2t+1
            for t in range(NPAIR):
                ctxu_sb.append(qpool.tile([128, W], BF16, tag="ctxu",
                                          name=f"ctxu{t}"))
            rsc = dpool.tile([NPAIR, 2 * W], BF16, name="rsc")
            dnd = dpool.tile([NPAIR, 2 * W], F32, name="dnd")

            def normalize(t):
                # broadcast 1/denom (staged in DRAM) across partitions, then
                # scale ctxu in place.  Called one pair late so nothing here
                # is on the critical path.
                bc = mpool.tile([128, W], BF16, tag="bcast", name=f"bc{t}")
                src = bass.AP(tensor=rsc.tensor,
                              offset=rsc.offset + t * 2 * W,
                              ap=[[W, 2], [0, DH], [1, W]])
                nc.gpsimd.dma_start(out=bc, in_=src)
                nc.vector.tensor_mul(ctxu_sb[t], ctxu_sb[t], bc)

            emit_qp(0)
            emit_qp(1)
            for t in range(NPAIR):  # head pairs (2t, 2t+1)
                qpt = qpt_sb[t]
                ctxA = psacc.tile([128, W], F32, tag="acc", name=f"ctxA{t}")
                ctxB = psacc.tile([128, W], F32, tag="acc", name=f"ctxB{t}")
                qps = None
                if t + 2 < NPAIR:
                    qps = psacc.tile([128, W], F32, tag="acc",
                                     name=f"qps{t + 2}")
                for j in range(NJ):
                    qk = psmm.tile([128, 1024], F32, tag="mm", name=f"qk{t}_{j}")
                    nc.tensor.matmul(qk[:, 0:W],
                                     kbig[0:DH, 128 * j:128 * (j + 1)],
                                     qpt[0:DH, :], start=True, stop=True)
                    nc.tensor.matmul(qk[:, W:2 * W],
                                     kbig[DH:2 * DH, 128 * j:128 * (j + 1)],
                                     qpt[DH:128, :], start=True, stop=True)
                    pr = ppool.tile([128, 1024], BF16, tag="probs",
                                    name=f"pr{t}_{j}")
                    nc.scalar.activation(pr, qk, AF.Exp, scale=0.125)
                    nc.tensor.matmul(ctxA[0:DH + 1, :], vbig[:, j, 0:DH + 1],
                                     pr[:, 0:W],
                                     start=(j == 0), stop=(j == NJ - 1))
                    nc.tensor.matmul(ctxB[0:DH + 1, :], vbig[:, j, 0:DH + 1],
                                     pr[:, W:2 * W],
                                     start=(j == 0), stop=(j == NJ - 1))
                    # ride the next-next pair's QP_T matmuls in ACT's slack
                    if qps is not None and j < DMT:
                        emit_qp_mm(qps, t + 2, j)
                # context (cast bf16) + reciprocal of the denominator rows
                nc.vector.tensor_copy(ctxu_sb[t][0:DH, :], ctxA[0:DH, :])
                nc.vector.tensor_copy(ctxu_sb[t][DH:128, :], ctxB[0:DH, :])
                # denominators: bounce through DRAM to respread [1, 1024]
                # over 32 partitions (single-partition DVE ops are ~6us).
                dtmp = mpool.tile([1, 2 * W], F32, tag="dtmp", name=f"dtmp{t}",
                                  bufs=2)
                nc.vector.tensor_copy(dtmp[:, 0:W], ctxA[DH:DH + 1, :])
                nc.vector.tensor_copy(dtmp[:, W:2 * W], ctxB[DH:DH + 1, :])
                nc.sync.dma_start(out=dnd[t:t + 1, :], in_=dtmp)
                sq = mpool.tile([32, 32], F32, tag="sq", name=f"sq{t}", bufs=2)
                srcd = bass.AP(tensor=dnd.tensor, offset=dnd.offset + t * 2 * W,
                               ap=[[32, 32], [1, 32]])
                nc.gpsimd.dma_start(out=sq, in_=srcd)
                rq = mpool.tile([32, 32], F32, tag="rq", name=f"rq{t}", bufs=2)
                nc.vector.reciprocal(rq, sq)
                rq16 = mpool.tile([32, 32], BF16, tag="rq16", name=f"rq16{t}",
                                  bufs=2)
                nc.vector.tensor_copy(rq16, rq)
                nc.sync.dma_start(out=rsc[t:t + 1, :], in_=rq16)
                if qps is not None:
                    sb = qpool.tile([128, LS], BF16, tag="qpt",
                                    name=f"qpt{t + 2}")
                    nc.vector.tensor_copy(sb, qps)
                    qpt_sb[t + 2] = sb
                # deferred normalization last: its broadcast DMA has had a
                # full pair to land, so the DVE never stalls here.
                if t > 0:
                    normalize(t - 1)

            # ---------- out = ctx @ Wc ----------
            # All four row blocks accumulate he chunks 0..6 while the last
            # pair's normalization chain completes (lt2/lt3 use PSUM
            # half-bank tiles from psacc); outputs evacuate on the
            # otherwise-idle scalar engine.
            wc_ps = {}  # (lt, half) -> psum AP [128, 512]
            for lt in (0, 1):
                full = psmm.tile([128, 1024], F32, tag="mm", name=f"wcp{lt}")
                wc_ps[(lt, 0)] = full[:, 0:512]
                wc_ps[(lt, 1)] = full[:, 512:1024]
            for lt in (2, 3):
                for half in range(2):
                    wc_ps[(lt, half)] = psacc.tile([128, W], F32, tag="acc",
                                                   name=f"wcp{lt}_{half}")

            def emit_wc(lt, he_list):
                for half in range(2):
                    for he in he_list:
                        nc.tensor.matmul(
                            wc_ps[(lt, half)],
                            ctxu_sb[he][:, 128 * lt:128 * (lt + 1)],
                            wc_sb[:, he, 512 * half:512 * (half + 1)],
                            start=(he == 0), stop=(he == DMT - 1))

            def emit_out(lt):
                ob = mpool.tile([128, DM], F32, tag="outsb", bufs=2)
                nc.scalar.activation(ob[:, 0:512], wc_ps[(lt, 0)], AF.Copy)
                nc.scalar.activation(ob[:, 512:1024], wc_ps[(lt, 1)], AF.Copy)
                ENGS[lt % 3].dma_start(out=out[128 * lt:128 * (lt + 1), :],
                                       in_=ob)

            for lt in range(NLT):
                emit_wc(lt, range(7))
            normalize(NPAIR - 1)
            for lt in range(NLT):
                emit_wc(lt, [7])
                emit_out(lt)

    nc.compile()
    return nc


_NC = None


def _get_nc():
    global _NC
    if _NC is None:
        _NC = build_nc()
    return _NC


def prep_in_maps(q, kv, Wq, Wkv, Wc):
    """Host-side input prep: transpose, cast to bf16, shard queries."""
    bf16 = ml_dtypes.bfloat16
    qT_full = np.ascontiguousarray(np.asarray(q, dtype=np.float32)[0].T
                                   ).astype(bf16)
    kvT = np.ascontiguousarray(np.asarray(kv, dtype=np.float32)[0].T
                               ).astype(bf16)
    Wq = np.ascontiguousarray(np.asarray(Wq, dtype=np.float32)).astype(bf16)
    Wkv = np.ascontiguousarray(np.asarray(Wkv, dtype=np.float32)).astype(bf16)
    Wc = np.ascontiguousarray(np.asarray(Wc, dtype=np.float32)).astype(bf16)
    in_maps = []
    for i in range(N_CORES):
        in_maps.append({
            "qT": np.ascontiguousarray(qT_full[:, LS * i:LS * (i + 1)]),
            "kvT": kvT,
            "Wq": Wq,
            "Wkv": Wkv,
            "Wc": Wc,
        })
    return in_maps


def kernel(q, kv, Wq, Wkv, Wc, w):
    assert int(w) == W
    q = np.asarray(q, dtype=np.float32)
    B = q.shape[0]
    assert B == 1 and q.shape[1] == L and q.shape[2] == DM

    in_maps = prep_in_maps(q, kv, Wq, Wkv, Wc)
    nc = _get_nc()
    res = run_bass_kernel_spmd(nc, in_maps, list(range(N_CORES)))
    out = np.concatenate([res.results[i]["out"] for i in range(N_CORES)], axis=0)
    return out.reshape(1, L, DM).astype(np.float32)
